# revision 9
# baseline (speedup 1.0000x reference)
"""Self-contained Trainium2 Bass kernel for nn_MultiHeadAttention_75273596829862.

Sharding: 8 cores = 2 batches x 4 head-groups (3 heads each). Each core
computes QKV projections for its heads, transposed-softmax attention, and a
partial output projection out_pT[768,2048]; the host sums 4 partials per
batch and adds bo.
"""

import sys

for p in ("/opt/trn_rl_repo", "/root/.axon_site/_ro/trn_rl_repo"):
    if p not in sys.path:
        sys.path.insert(0, p)

import numpy as np
import ml_dtypes

BF16 = ml_dtypes.bfloat16

# Problem constants (hardcoded per spec)
B, S, DM = 2, 2048, 768
H, HD = 12, 64
NCORES = 8
HPC = 3          # heads per core
DQK = 2 * HPC * HD  # 384: packed Q/K projection output columns
KT = DM // 128   # 6 k-tiles of the contraction dim
ST = S // 128    # 16 t-tiles of the sequence
NC_CHUNK = 512   # matmul moving free-dim chunk
NCH = S // NC_CHUNK  # 4 chunks of the s dimension

_compiled = None  # cached (nc, names) after first build


def _build(loop_reps=0, dbg=False):
    import concourse.bass as bass
    import concourse.bacc as bacc
    import concourse.tile as tile
    import concourse.mybir as mybir
    from concourse.bass import ts, ds

    dt = mybir.dt
    AF = mybir.ActivationFunctionType

    nc = bacc.Bacc("TRN2", target_bir_lowering=False, debug=False)

    # DRAM I/O (per-core shard shapes)
    xqT = nc.dram_tensor("xqT", [DM, S], dt.bfloat16, kind="ExternalInput").ap()
    xkT = nc.dram_tensor("xkT", [DM, S], dt.bfloat16, kind="ExternalInput").ap()
    xvT = nc.dram_tensor("xvT", [DM, S], dt.bfloat16, kind="ExternalInput").ap()
    # wcat columns: [Q_h0|Q_h1 | K_h0|K_h1 | Q_h2|K_h2]
    wcat = nc.dram_tensor("wcat", [DM, DQK], dt.bfloat16, kind="ExternalInput").ap()
    wv = nc.dram_tensor("wv", [DM, HPC * HD], dt.bfloat16, kind="ExternalInput").ap()
    # wo rows for this core's heads: [192, 768]
    wo = nc.dram_tensor("wo", [HPC * HD, DM], dt.bfloat16, kind="ExternalInput").ap()
    # bcat: biases matching wcat column order, [384] -> [128,3]
    bcat = nc.dram_tensor("bcat", [DQK], dt.float32, kind="ExternalInput").ap()
    bv = nc.dram_tensor("bv", [HPC * HD], dt.float32, kind="ExternalInput").ap()
    out_pT = nc.dram_tensor("out_pT", [DM, S], dt.bfloat16, kind="ExternalOutput").ap()
    dbg_t = {}
    if dbg:
        for nm, shp, dty in [("d_qk01", [128, S], dt.bfloat16),
                             ("d_k01", [128, S], dt.bfloat16),
                             ("d_qk2", [128, S], dt.bfloat16),
                             ("d_kT2", [HD, S], dt.bfloat16),
                             ("d_vv0", [128, HD + 1], dt.bfloat16),
                             ("d_exp0", [128, S], dt.bfloat16),
                             ("d_po0", [HD + 1, NC_CHUNK], dt.float32),
                             ("d_rec0", [1, NC_CHUNK], dt.float32),
                             ("d_bc0", [HD, NC_CHUNK], dt.float32),
                             ("d_on0", [HD, S], dt.bfloat16)]:
            dbg_t[nm] = nc.dram_tensor(nm, shp, dty, kind="ExternalOutput").ap()

    with tile.TileContext(nc) as tc:
        def body():
            with tc.tile_pool(name="consts", bufs=1) as cpool:
                # weights resident in SBUF
                wcat_sb = []
                wv_sb = []
                for k in range(KT):
                    t1 = cpool.tile([128, DQK], dt.bfloat16, tag=f"wcat{k}")
                    nc.sync.dma_start(out=t1, in_=wcat[ts(k, 128), :])
                    wcat_sb.append(t1)
                    t2 = cpool.tile([128, HPC * HD], dt.bfloat16, tag=f"wv{k}")
                    nc.sync.dma_start(out=t2, in_=wv[ts(k, 128), :])
                    wv_sb.append(t2)
                wo_sb = []
                for h in range(HPC):
                    t3 = cpool.tile([HD, DM], dt.bfloat16, tag=f"wo{h}")
                    nc.sync.dma_start(out=t3, in_=wo[ts(h, HD), :])
                    wo_sb.append(t3)
                bqk_sb = cpool.tile([128, HPC], dt.float32, tag="bqk")
                nc.sync.dma_start(
                    out=bqk_sb, in_=bcat.rearrange("(m p) -> p m", p=128)
                )
                bv_sb = cpool.tile([HD, HPC], dt.float32, tag="bv")
                nc.sync.dma_start(out=bv_sb, in_=bv.rearrange("(h d) -> d h", d=HD))

                # xvT fully resident (needed as [128,128] stationaries)
                xvT_sb = []
                for k in range(KT):
                    t4 = cpool.tile([128, S], dt.bfloat16, tag=f"xvT{k}")
                    nc.sync.dma_start(out=t4, in_=xvT[ts(k, 128), :])
                    xvT_sb.append(t4)

                # per-head Q/K storage (see column packing above)
                qk01_sb = cpool.tile([128, S], dt.bfloat16, tag="qk01")  # Q_h0|Q_h1
                k01_sb = cpool.tile([128, S], dt.bfloat16, tag="k01")    # K_h0|K_h1
                qk2_sb = cpool.tile([128, S], dt.bfloat16, tag="qk2")    # Q_h2|K_h2raw
                kT2_sb = cpool.tile([HD, S], dt.bfloat16, tag="kT2")     # K_h2 at base 0
                # V~ tiles [128,65] per (h,t): V normal + ones column
                vv_sb = [
                    [cpool.tile([128, HD + 1], dt.bfloat16, tag=f"vv{h}_{t}", name=f"vv{h}_{t}")
                     for t in range(ST)]
                    for h in range(HPC)
                ]
                # normalized attention outputs per head [64, 2048] bf16
                onrm_sb = [
                    cpool.tile([HD, S], dt.bfloat16, tag=f"onrm{h}", name=f"onrm{h}")
                    for h in range(HPC)
                ]

                # ---------------- QK projections ----------------
                with (
                    tc.tile_pool(name="xs", bufs=2) as xpool,
                    tc.tile_pool(name="pproj", bufs=2, space="PSUM") as pqk,
                ):
                    for c in range(NCH):
                        xq_t = []
                        xk_t = []
                        for k in range(KT):
                            tq = xpool.tile([128, NC_CHUNK], dt.bfloat16,
                                            tag=f"xq{k}")
                            nc.sync.dma_start(out=tq,
                                              in_=xqT[ts(k, 128), ts(c, NC_CHUNK)])
                            xq_t.append(tq)
                            tk = xpool.tile([128, NC_CHUNK], dt.bfloat16,
                                            tag=f"xk{k}")
                            nc.sync.dma_start(out=tk,
                                              in_=xkT[ts(k, 128), ts(c, NC_CHUNK)])
                            xk_t.append(tk)
                        # 4 matmul groups -> 3 psum tiles
                        groups = [
                            (0, 128, xq_t, qk01_sb, 0),   # Q_h0|Q_h1
                            (128, 128, xk_t, k01_sb, 1),  # K_h0|K_h1
                            (256, 64, xq_t, qk2_sb, 2),   # Q_h2 (rows 0:64)
                            (320, 64, xk_t, qk2_sb, 2),   # K_h2 (rows 64:128)
                        ]
                        pt_cache = {}
                        for col0, m, xsrc, dst, bi in groups:
                            if bi not in pt_cache:
                                pt_cache[bi] = pqk.tile([128, NC_CHUNK], dt.float32,
                                                        tag=f"pqk{bi}", name=f"pqk{bi}")
                            pt = pt_cache[bi]
                            row0 = 0 if (col0 != 320) else 64
                            for k in range(KT):
                                nc.tensor.matmul(
                                    pt[row0:row0 + m, :],
                                    wcat_sb[k][:, ds(col0, m)],
                                    xsrc[k],
                                    start=(k == 0),
                                    stop=(k == KT - 1),
                                )
                        # psum -> SBUF with bias add (per-partition scalar)
                        for bi, dst in ((0, qk01_sb), (1, k01_sb), (2, qk2_sb)):
                            nc.vector.tensor_scalar_add(
                                dst[:, ts(c, NC_CHUNK)],
                                pt_cache[bi],
                                bqk_sb[:, bi:bi + 1],
                            )

                # ------------- V projection (normal layout) -------------
                with tc.tile_pool(name="pv", bufs=2, space="PSUM") as pv:
                    for t in range(ST):
                        pvt = [pv.tile([128, HD], dt.float32, tag=f"pvh{h}", name=f"pvh{h}")
                               for h in range(HPC)]
                        for k in range(KT):
                            for h in range(HPC):
                                nc.tensor.matmul(
                                    pvt[h],
                                    xvT_sb[k][:, ts(t, 128)],
                                    wv_sb[k][:, ts(h, HD)],
                                    start=(k == 0),
                                    stop=(k == KT - 1),
                                )
                        for h in range(HPC):
                            nc.vector.tensor_copy(vv_sb[h][t][:, 0:HD], pvt[h])
                            nc.gpsimd.memset(vv_sb[h][t][:, HD:HD + 1], 1.0)

                # K_h2 lives at partitions 64:128 of qk2_sb; move to base 0
                nc.sync.dma_start(out=kT2_sb, in_=qk2_sb[64:128, :])

                qk_heads = [
                    (qk01_sb[0:64, :], k01_sb[0:64, :]),
                    (qk01_sb[64:128, :], k01_sb[64:128, :]),
                    (qk2_sb[0:64, :], kT2_sb),
                ]

                # ---------------- attention ----------------
                with (
                    tc.tile_pool(name="ps", bufs=1, space="PSUM") as psp,
                    tc.tile_pool(name="po", bufs=2, space="PSUM") as pop,
                    tc.tile_pool(name="pout", bufs=1, space="PSUM") as poutp,
                    tc.tile_pool(name="exps", bufs=16) as epool,
                    tc.tile_pool(name="smalls", bufs=4) as spool,
                    tc.tile_pool(name="outs", bufs=2) as opool,
                ):
                    for h in range(HPC):
                        qT_h, kT_h = qk_heads[h]
                        # scores^T + exp, per t-tile over full s
                        exp_t = []
                        for t in range(ST):
                            ps = psp.tile([128, S], dt.float32, tag="ps")
                            for c in range(NCH):
                                nc.tensor.matmul(
                                    ps[:, ts(c, NC_CHUNK)],
                                    kT_h[:, ts(t, 128)],
                                    qT_h[:, ts(c, NC_CHUNK)],
                                )
                            et = epool.tile([128, S], dt.bfloat16, tag="exp")
                            nc.scalar.activation(et, ps, AF.Exp,
                                                 scale=1.0 / np.sqrt(HD))
                            if dbg and h == 0 and t == 0:
                                nc.sync.dma_start(out=dbg_t["d_exp0"], in_=et)
                            exp_t.append(et)
                        # PV per s-chunk: accumulate over t; row 64 = denom
                        for c in range(NCH):
                            po = pop.tile([HD + 1, NC_CHUNK], dt.float32, tag="po")
                            for t in range(ST):
                                nc.tensor.matmul(
                                    po,
                                    vv_sb[h][t],
                                    exp_t[t][:, ts(c, NC_CHUNK)],
                                    start=(t == 0),
                                    stop=(t == ST - 1),
                                )
                            # denominator -> SBUF row, reciprocal, broadcast
                            dtile = spool.tile([1, NC_CHUNK], dt.float32, tag="den")
                            nc.vector.tensor_copy(dtile, po[HD:HD + 1, :])
                            rtile = spool.tile([1, NC_CHUNK], dt.float32, tag="rec")
                            nc.vector.reciprocal_approx_fast(out=rtile, in_=dtile)
                            bcast = spool.tile([HD, NC_CHUNK], dt.float32,
                                               tag="bcast")
                            nc.gpsimd.partition_broadcast(bcast, rtile)
                            # normalize + bias -> bf16
                            dst = onrm_sb[h][:, ts(c, NC_CHUNK)]
                            nc.vector.tensor_mul(dst, po[0:HD, :], bcast)
                            nc.vector.tensor_scalar_add(dst, dst,
                                                        bv_sb[:, h:h + 1])
                            if dbg and h == 0 and c == 0:
                                pos = spool.tile([HD + 1, NC_CHUNK], dt.float32,
                                                 tag="dpo")
                                nc.vector.tensor_copy(pos, po)
                                nc.sync.dma_start(out=dbg_t["d_po0"], in_=pos)
                                nc.sync.dma_start(out=dbg_t["d_rec0"], in_=rtile)
                                nc.sync.dma_start(out=dbg_t["d_bc0"], in_=bcast)

                    if dbg:
                        nc.sync.dma_start(out=dbg_t["d_qk01"], in_=qk01_sb)
                        nc.sync.dma_start(out=dbg_t["d_k01"], in_=k01_sb)
                        nc.sync.dma_start(out=dbg_t["d_qk2"], in_=qk2_sb)
                        nc.sync.dma_start(out=dbg_t["d_kT2"], in_=kT2_sb)
                        nc.sync.dma_start(out=dbg_t["d_vv0"], in_=vv_sb[0][0])
                        nc.sync.dma_start(out=dbg_t["d_on0"], in_=onrm_sb[0])
                    # ---------------- output projection ----------------
                    for e in range(KT):
                        for c in range(NCH):
                            pout = poutp.tile([128, NC_CHUNK], dt.float32,
                                              tag="pout")
                            for h in range(HPC):
                                nc.tensor.matmul(
                                    pout,
                                    wo_sb[h][:, ts(e, 128)],
                                    onrm_sb[h][:, ts(c, NC_CHUNK)],
                                    start=(h == 0),
                                    stop=(h == HPC - 1),
                                )
                            ot = opool.tile([128, NC_CHUNK], dt.bfloat16,
                                            tag="ot")
                            nc.vector.tensor_copy(ot, pout)
                            nc.sync.dma_start(
                                out=out_pT[ts(e, 128), ts(c, NC_CHUNK)], in_=ot
                            )

        if loop_reps > 1:
            with tc.For_i(0, loop_reps, 1):
                body()
        else:
            body()

    nc.compile()
    return nc


def _shard_inputs(query, key, value, wq, bq, wk, bk, wv, bv, wo, bo):
    """Build the 8 per-core input maps."""
    in_maps = []
    for core in range(NCORES):
        b = core // 4
        h0 = (core % 4) * HPC
        cs = slice(h0 * HD, (h0 + HPC) * HD)
        wq_s, wk_s, wv_s = wq[:, cs], wk[:, cs], wv[:, cs]
        # wcat columns: [Q_h0|Q_h1 | K_h0|K_h1 | Q_h2|K_h2]
        wcat = np.concatenate(
            [wq_s[:, 0:128], wk_s[:, 0:128], wq_s[:, 128:192], wk_s[:, 128:192]],
            axis=1,
        )
        bq_s, bk_s, bv_s = bq[cs], bk[cs], bv[cs]
        bcat = np.concatenate([bq_s[0:128], bk_s[0:128], bq_s[128:192],
                               bk_s[128:192]])
        in_maps.append({
            "xqT": np.ascontiguousarray(query[b].T).astype(BF16),
            "xkT": np.ascontiguousarray(key[b].T).astype(BF16),
            "xvT": np.ascontiguousarray(value[b].T).astype(BF16),
            "wcat": np.ascontiguousarray(wcat).astype(BF16),
            "wv": np.ascontiguousarray(wv_s).astype(BF16),
            "wo": np.ascontiguousarray(wo[cs, :]).astype(BF16),
            "bcat": np.ascontiguousarray(bcat).astype(np.float32),
            "bv": np.ascontiguousarray(bv_s).astype(np.float32),
        })
    return in_maps


def kernel(query, key, value, wq, bq, wk, bk, wv, bv, wo, bo):
    global _compiled
    from concourse.bass_utils import run_bass_kernel_spmd

    if _compiled is None:
        _compiled = _build()
    nc = _compiled

    in_maps = _shard_inputs(query, key, value, wq, bq, wk, bk, wv, bv, wo, bo)
    res = run_bass_kernel_spmd(nc, in_maps, list(range(NCORES)))

    out = np.zeros((B, S, DM), dtype=np.float32)
    for core in range(NCORES):
        b = core // 4
        out[b] += res.results[core]["out_pT"].astype(np.float32).T
    out += np.asarray(bo, dtype=np.float32)[None, None, :]
    return out
